# revision 6
# baseline (speedup 1.0000x reference)
"""GCN message-passing kernel for Trainium2, 8 NeuronCores (SPMD).

Strategy (dest-sharded graph parallelism):
  - Nodes sharded 8 ways; edges partitioned by destination shard, sorted by dest.
  - Per layer: each core gathers transformed source features (indirect DMA, one
    128-row chunk per call) for its edges, aggregates via one-hot matmuls into
    PSUM (segment-sum), computes BN stats on its shard, all-gathers stats (tiny)
    and the next layer's gather table (node-major x@W), so every core has the
    full table for the next layer's gathers.
  - Layer 1 aggregates raw x (32-dim) and applies W1 afterwards; layers 2-4
    aggregate pre-transformed features.
  - Decoder: u|v table (enc @ dw1 halves, +bias folded) built from the
    all-gathered encoder output; per-label gathers + small MLP, feature-major
    via PE transposes.

kernel(**inputs) accepts FULL inputs, returns the FULL [N_LABEL] output.
"""
import math
import numpy as np

import concourse.bass as bass
import concourse.bacc as bacc
import concourse.mybir as mybir
import concourse.tile as tile
from concourse.bass_utils import run_bass_kernel_spmd

F32 = mybir.dt.float32
I32 = mybir.dt.int32
AF = mybir.ActivationFunctionType
OP = mybir.AluOpType
AX = mybir.AxisListType

NCORES = 8
BN_EPS = 1e-5
P = 128


# ---------------------------------------------------------------------------
# Host-side preparation
# ---------------------------------------------------------------------------

def host_prep(x, edge_index, edge_weight, n_nodes):
    n = n_nodes
    shard = n // NCORES
    nt = math.ceil(shard / P)
    srow = nt * P

    rows = np.concatenate([edge_index[0].astype(np.int64), np.arange(n, dtype=np.int64)])
    cols = np.concatenate([edge_index[1].astype(np.int64), np.arange(n, dtype=np.int64)])
    ew = np.concatenate([np.asarray(edge_weight, np.float64), np.ones(n, np.float64)])

    deg = np.bincount(cols, weights=ew, minlength=n)
    dinv = np.where(deg > 0, deg ** -0.5, 0.0)
    norm = (dinv[rows] * ew * dinv[cols]).astype(np.float32)

    order = np.argsort(cols, kind="stable")
    rows_s, cols_s, norm_s = rows[order], cols[order], norm[order]

    core_of = cols_s // shard
    loc_all = cols_s - core_of * shard
    t_all = loc_all // P
    counts = np.zeros((NCORES, nt), np.int64)
    for c in range(NCORES):
        counts[c] = np.bincount(t_all[core_of == c], minlength=nt)
    tpt = np.maximum(1, np.ceil(counts.max(axis=0) / P).astype(np.int64))
    C = int(tpt.sum())
    base = np.concatenate([[0], np.cumsum(tpt)])

    tab_row = (rows_s // shard) * srow + (rows_s % shard)

    ridx = np.zeros((NCORES, P, C), np.int32)
    colf = np.full((NCORES, P, C), -1.0, np.float32)
    nrmf = np.zeros((NCORES, P, C), np.float32)
    for c in range(NCORES):
        m = core_of == c
        t = t_all[m]
        starts = np.searchsorted(t, np.arange(nt))
        k = np.arange(t.size) - starts[t]
        chunk = base[t] + k // P
        lane = k % P
        ridx[c][lane, chunk] = tab_row[m]
        colf[c][lane, chunk] = (loc_all[m] - t * P).astype(np.float32)
        nrmf[c][lane, chunk] = norm_s[m]

    fx = x.shape[1]
    x_pad = np.zeros((NCORES * srow, fx), np.float32)
    for c in range(NCORES):
        x_pad[c * srow:c * srow + shard] = x[c * shard:(c + 1) * shard]

    return dict(shard=shard, nt=nt, srow=srow, C=C, tpt=[int(v) for v in tpt],
                ridx=ridx, colf=colf, nrmf=nrmf, x_pad=x_pad)


# ---------------------------------------------------------------------------
# Device program
# ---------------------------------------------------------------------------

def build_program(cfg, skip_cc=False, reps=1):
    nt, srow, C, tpt = cfg["nt"], cfg["srow"], cfg["C"], cfg["tpt"]
    fx = cfg["fx"]
    NLC = cfg["n_label_core"]
    LCH = NLC // P
    n_nodes = cfg["n_nodes"]
    NTAB = NCORES * srow
    HG = NCORES // 2                      # uv build: cores per half

    nc = bacc.Bacc(None, num_devices=NCORES)

    x_tab = nc.dram_tensor("x_tab", [NTAB, fx], F32, kind="ExternalInput")
    ridx = nc.dram_tensor("ridx", [P, C], I32, kind="ExternalInput")
    colf = nc.dram_tensor("colf", [P, C], F32, kind="ExternalInput")
    nrmf = nc.dram_tensor("nrmf", [P, C], F32, kind="ExternalInput")
    aidx = nc.dram_tensor("aidx", [P, LCH], I32, kind="ExternalInput")
    bidx = nc.dram_tensor("bidx", [P, LCH], I32, kind="ExternalInput")
    w1 = nc.dram_tensor("w1", [fx, P], F32, kind="ExternalInput")
    w2 = nc.dram_tensor("w2", [P, P], F32, kind="ExternalInput")
    w3 = nc.dram_tensor("w3", [P, P], F32, kind="ExternalInput")
    w4 = nc.dram_tensor("w4", [P, 64], F32, kind="ExternalInput")
    bn_g = nc.dram_tensor("bn_g", [3, P], F32, kind="ExternalInput")
    bn_b = nc.dram_tensor("bn_b", [3, P], F32, kind="ExternalInput")
    duv = nc.dram_tensor("duv", [65, P], F32, kind="ExternalInput")
    dw2 = nc.dram_tensor("dw2", [64, 64], F32, kind="ExternalInput")
    db2 = nc.dram_tensor("db2", [64, 1], F32, kind="ExternalInput")
    dw3 = nc.dram_tensor("dw3", [64, 1], F32, kind="ExternalInput")
    db3 = nc.dram_tensor("db3", [1, 1], F32, kind="ExternalInput")
    iota_in = nc.dram_tensor("iota", [P, P], F32, kind="ExternalInput")
    ident_in = nc.dram_tensor("ident", [P, P], F32, kind="ExternalInput")
    out = nc.dram_tensor("out", [1, NLC], F32, kind="ExternalOutput")

    rg = [list(range(NCORES))]

    with tile.TileContext(nc) as tc:
        with (
            tc.tile_pool(name="const", bufs=1) as constp,
            tc.tile_pool(name="msg", bufs=6) as msgp,
            tc.tile_pool(name="oh", bufs=6) as ohp,
            tc.tile_pool(name="big", bufs=1) as bigp,
            tc.tile_pool(name="small", bufs=2) as smallp,
            tc.tile_pool(name="psA", bufs=3, space="PSUM") as psA,
            tc.tile_pool(name="psB", bufs=4, space="PSUM") as psB,
            tc.tile_pool(name="dram", bufs=1, space="DRAM") as dram,
        ):
            def load_const(src, shape, tag, dtype=F32):
                t = constp.tile(shape, dtype, tag=tag, name=tag)
                nc.sync.dma_start(out=t[:], in_=src)
                return t

            ridx_t = load_const(ridx[:], [P, C], "ridx", I32)
            colf_t = load_const(colf[:], [P, C], "colf")
            nrmf_t = load_const(nrmf[:], [P, C], "nrmf")
            iota_t = load_const(iota_in[:], [P, P], "iota")
            ident_t = load_const(ident_in[:], [P, P], "ident")
            w1_t = load_const(w1[:], [fx, P], "w1")
            w2_t = load_const(w2[:], [P, P], "w2")
            w3_t = load_const(w3[:], [P, P], "w3")
            w4_t = load_const(w4[:], [P, 64], "w4")
            dw2_t = load_const(dw2[:], [64, 64], "dw2")
            db2_t = load_const(db2[:], [64, 1], "db2")
            dw3_t = load_const(dw3[:], [64, 1], "dw3")
            db3_t = load_const(db3[:], [1, 1], "db3")
            aidx_t = load_const(aidx[:], [P, LCH], "aidx", I32)
            bidx_t = load_const(bidx[:], [P, LCH], "bidx", I32)
            duv_t = load_const(duv[:], [65, P], "duv")
            bn_cols = []
            for l in range(3):
                g = load_const(bn_g[l:l + 1, :], [P, 1], f"bng{l}")
                b = load_const(bn_b[l:l + 1, :], [P, 1], f"bnb{l}")
                bn_cols.append((g, b))

            tabs = {
                2: dram.tile([NTAB * P], F32, addr_space="Shared", tag="tab2", name="tab2"),
                3: dram.tile([NTAB * P], F32, addr_space="Shared", tag="tab3", name="tab3"),
                4: dram.tile([NTAB * 64], F32, addr_space="Shared", tag="tab4", name="tab4"),
            }
            enc_ag = dram.tile([NCORES * 64 * srow], F32, addr_space="Shared", tag="encag")
            uv_tab = dram.tile([NTAB, P], F32, tag="uvtab")
            cc_xw = dram.tile([srow * P], F32, tag="ccxw")
            cc_enc = dram.tile([64 * srow], F32, tag="ccenc")
            cc_st_in = dram.tile([P * 2], F32, tag="stin")
            cc_st_outs = [dram.tile([NCORES * P * 2], F32, addr_space="Shared",
                                    tag=f"stout{i}", name=f"stout{i}") for i in range(3)]

            loop_ctx = tc.For_i(0, reps, 1) if reps > 1 else None
            if loop_ctx is not None:
                loop_ctx.__enter__()

            # ---------------- aggregation ----------------
            def aggregate(table_ap, fm):
                agg_sb = bigp.tile([fm, nt * P], F32, tag="agg")
                c = 0
                for t in range(nt):
                    pt = psA.tile([fm, P], F32, space="PSUM", tag="agg")
                    for j in range(tpt[t]):
                        m = msgp.tile([P, fm], F32, tag="msg")
                        nc.gpsimd.indirect_dma_start(
                            out=m[:], out_offset=None, in_=table_ap,
                            in_offset=bass.IndirectOffsetOnAxis(
                                ap=ridx_t[:, c:c + 1], axis=0))
                        oh = ohp.tile([P, P], F32, tag="oh")
                        nc.vector.tensor_scalar(
                            out=oh[:], in0=iota_t[:],
                            scalar1=colf_t[:, c:c + 1], scalar2=nrmf_t[:, c:c + 1],
                            op0=OP.is_equal, op1=OP.mult)
                        nc.tensor.matmul(out=pt[:], lhsT=m[:], rhs=oh[:],
                                         start=(j == 0), stop=(j == tpt[t] - 1))
                        c += 1
                    nc.scalar.activation(out=agg_sb[:, t * P:(t + 1) * P], in_=pt[:],
                                         func=AF.Copy)
                return agg_sb

            # ---------------- batchnorm + relu ----------------
            def bn_relu(s_sb, layer_idx):
                cc_st_out = cc_st_outs[layer_idx]
                sums = smallp.tile([P, 2], F32, tag="sums")
                nc.vector.reduce_sum(out=sums[:, 0:1], in_=s_sb[:], axis=AX.X)
                sq = bigp.tile([P, nt * P], F32, tag="h")  # scratch in h slot
                nc.vector.tensor_tensor(out=sq[:], in0=s_sb[:], in1=s_sb[:], op=OP.mult)
                nc.vector.reduce_sum(out=sums[:, 1:2], in_=sq[:], axis=AX.X)
                nc.sync.dma_start(
                    out=cc_st_in[:].rearrange("(p s) -> p s", p=P), in_=sums[:])
                if not skip_cc:
                    nc.gpsimd.collective_compute(
                        "AllGather", OP.bypass, replica_groups=rg,
                        ins=[cc_st_in[:]], outs=[cc_st_out[:]])
                allst = smallp.tile([P, 2, NCORES], F32, tag="allst")
                nc.sync.dma_start(
                    out=allst[:],
                    in_=cc_st_out[:].rearrange("(g p s) -> p s g", p=P, s=2))
                tot = smallp.tile([P, 2], F32, tag="tot")
                nc.vector.reduce_sum(out=tot[:], in_=allst[:], axis=AX.X)
                mv = smallp.tile([P, 4], F32, tag="mv")
                nc.vector.tensor_scalar_mul(mv[:, 0:2], tot[:], 1.0 / float(n_nodes))
                nc.vector.tensor_tensor(out=mv[:, 2:3], in0=mv[:, 0:1], in1=mv[:, 0:1],
                                        op=OP.mult)
                nc.vector.tensor_tensor(out=mv[:, 2:3], in0=mv[:, 1:2], in1=mv[:, 2:3],
                                        op=OP.subtract)
                nc.vector.tensor_scalar_add(mv[:, 3:4], mv[:, 2:3], BN_EPS)
                rstd = smallp.tile([P, 1], F32, tag="rstd")
                nc.scalar.activation(out=rstd[:], in_=mv[:, 3:4], func=AF.Sqrt)
                nc.vector.reciprocal(out=rstd[:], in_=rstd[:])
                g_col, b_col = bn_cols[layer_idx]
                scale = smallp.tile([P, 1], F32, tag="scale")
                nc.vector.tensor_tensor(out=scale[:], in0=g_col[:], in1=rstd[:], op=OP.mult)
                bias = smallp.tile([P, 1], F32, tag="bias")
                nc.vector.tensor_tensor(out=bias[:], in0=mv[:, 0:1], in1=scale[:], op=OP.mult)
                nc.vector.tensor_tensor(out=bias[:], in0=b_col[:], in1=bias[:], op=OP.subtract)
                h_sb = bigp.tile([P, nt * P], F32, tag="h")
                nc.scalar.activation(out=h_sb[:], in_=s_sb[:], func=AF.Relu,
                                     bias=bias[:, 0:1], scale=scale[:, 0:1])
                return h_sb

            # ---------------- next-layer table ----------------
            def build_table(h_sb, w_t, fout, tab):
                xw_sb = bigp.tile([P, nt * fout], F32, tag="tmp")
                for k in range(nt):
                    pxw = psB.tile([P, fout], F32, space="PSUM", tag="ps")
                    nc.tensor.matmul(out=pxw[:], lhsT=h_sb[:, k * P:(k + 1) * P],
                                     rhs=w_t[:], start=True, stop=True)
                    nc.scalar.activation(out=xw_sb[:, k * fout:(k + 1) * fout],
                                         in_=pxw[:], func=AF.Copy)
                nc.sync.dma_start(
                    out=cc_xw[0:srow * fout].rearrange("(k p f) -> p k f", p=P, k=nt),
                    in_=xw_sb[:].rearrange("p (k f) -> p k f", k=nt))
                if not skip_cc:
                    nc.gpsimd.collective_compute(
                        "AllGather", OP.bypass, replica_groups=rg,
                        ins=[cc_xw[0:srow * fout]], outs=[tab[0:NCORES * srow * fout]])

            # ======================= layers =======================
            agg1 = aggregate(x_tab[:], fx)
            s1_sb = bigp.tile([P, nt * P], F32, tag="tmp")
            nw = nt * P
            nsl = (nw + 511) // 512
            for k in range(nsl):
                c0 = k * 512
                c1 = min(nw, c0 + 512)
                ps1 = psB.tile([P, c1 - c0], F32, space="PSUM", tag="ps")
                nc.tensor.matmul(out=ps1[:], lhsT=w1_t[:], rhs=agg1[:, c0:c1],
                                 start=True, stop=True)
                nc.scalar.activation(out=s1_sb[:, c0:c1], in_=ps1[:], func=AF.Copy)
            h1 = bn_relu(s1_sb, 0)
            build_table(h1, w2_t, P, tabs[2])

            agg2 = aggregate(tabs[2][:].rearrange("(n f) -> n f", f=P), P)
            h2 = bn_relu(agg2, 1)
            build_table(h2, w3_t, P, tabs[3])

            agg3 = aggregate(tabs[3][:].rearrange("(n f) -> n f", f=P), P)
            h3 = bn_relu(agg3, 2)
            build_table(h3, w4_t, 64, tabs[4])

            agg4 = aggregate(tabs[4][:].rearrange("(n f) -> n f", f=64), 64)
            nc.sync.dma_start(out=cc_enc[:].rearrange("(f n) -> f n", f=64),
                              in_=agg4[:])
            if not skip_cc:
                nc.gpsimd.collective_compute(
                    "AllGather", OP.bypass, replica_groups=rg,
                    ins=[cc_enc[:]], outs=[enc_ag[:]])

            # ======================= u|v table =======================
            for g in range(NCORES):
                encf = bigp.tile([65, srow], F32, tag="encf")
                nc.vector.memset(encf[64:65, :], 1.0)
                nc.sync.dma_start(
                    out=encf[0:64, :],
                    in_=enc_ag[g * 64 * srow:(g + 1) * 64 * srow].rearrange(
                        "(f n) -> f n", f=64))
                uv_sb = bigp.tile([P, nt * P], F32, tag="tmp")
                for k in range(nt):
                    puv = psB.tile([P, P], F32, space="PSUM", tag="ps")
                    nc.tensor.matmul(out=puv[:], lhsT=encf[:, k * P:(k + 1) * P],
                                     rhs=duv_t[:], start=True, stop=True)
                    nc.scalar.activation(out=uv_sb[:, k * P:(k + 1) * P],
                                         in_=puv[:], func=AF.Copy)
                nc.sync.dma_start(
                    out=uv_tab[g * srow:(g + 1) * srow, :].rearrange(
                        "(k p) f -> p k f", p=P),
                    in_=uv_sb[:].rearrange("p (k f) -> p k f", k=nt))

            # ======================= decoder =======================
            SEG = min(16, LCH)
            z3_sb = None
            for lc in range(LCH):
                if lc % SEG == 0:
                    z3_sb = bigp.tile([1, SEG * P], F32, tag="z3", name="z3seg")
                ua = msgp.tile([P, 64], F32, tag="ua")
                nc.gpsimd.indirect_dma_start(
                    out=ua[:], out_offset=None, in_=uv_tab[:],
                    in_offset=bass.IndirectOffsetOnAxis(ap=aidx_t[:, lc:lc + 1], axis=0))
                vb = msgp.tile([P, 64], F32, tag="vb")
                nc.gpsimd.indirect_dma_start(
                    out=vb[:], out_offset=None, in_=uv_tab[:],
                    in_offset=bass.IndirectOffsetOnAxis(ap=bidx_t[:, lc:lc + 1], axis=0),
                    element_offset=64)
                z1 = msgp.tile([P, 64], F32, tag="z1")
                nc.vector.tensor_tensor(out=z1[:], in0=ua[:], in1=vb[:], op=OP.add)
                nc.scalar.activation(out=z1[:], in_=z1[:], func=AF.Relu)
                pz1t = psB.tile([64, P], F32, space="PSUM", tag="ps")
                nc.tensor.transpose(out=pz1t[:], in_=z1[:], identity=ident_t[:])
                z1t = msgp.tile([64, P], F32, tag="z1t")
                nc.vector.tensor_copy(out=z1t[:], in_=pz1t[:])
                pz2 = psB.tile([64, P], F32, space="PSUM", tag="ps")
                nc.tensor.matmul(out=pz2[:], lhsT=dw2_t[:], rhs=z1t[:],
                                 start=True, stop=True)
                z2 = msgp.tile([64, P], F32, tag="z2")
                nc.scalar.activation(out=z2[:], in_=pz2[:], func=AF.Relu,
                                     bias=db2_t[:, 0:1])
                pz3 = psB.tile([1, P], F32, space="PSUM", tag="ps")
                nc.tensor.matmul(out=pz3[:], lhsT=dw3_t[:], rhs=z2[:],
                                 start=True, stop=True)
                nc.vector.tensor_scalar_add(z3_sb[:, (lc % SEG) * P:(lc % SEG + 1) * P],
                                            pz3[:], db3_t[0:1, 0:1])
                if lc % SEG == SEG - 1:
                    seg0 = (lc - SEG + 1) * P
                    nc.sync.dma_start(out=out[:, seg0:seg0 + SEG * P], in_=z3_sb[:])

            if loop_ctx is not None:
                loop_ctx.__exit__(None, None, None)

    nc.finalize()
    return nc


# ---------------------------------------------------------------------------
# kernel entry
# ---------------------------------------------------------------------------

_CACHE = {}


def prepare(inputs, skip_cc=False, reps=1):
    x = np.asarray(inputs["x"], np.float32)
    edge_index = np.asarray(inputs["edge_index"])
    edge_weight = np.asarray(inputs["edge_weight"], np.float32)
    eli = np.asarray(inputs["edge_label_index"])
    n_nodes, fx = x.shape
    n_label = eli.shape[1]

    prep = host_prep(x, edge_index, edge_weight, n_nodes)
    shard, nt, srow, C = prep["shard"], prep["nt"], prep["srow"], prep["C"]
    nlc = n_label // NCORES

    cfg = dict(nt=nt, srow=srow, C=C, tpt=prep["tpt"], fx=fx,
               n_label_core=nlc, n_nodes=n_nodes)
    key = (nt, srow, C, tuple(prep["tpt"]), fx, nlc, n_nodes, skip_cc, reps)
    if key not in _CACHE:
        _CACHE[key] = build_program(cfg, skip_cc=skip_cc, reps=reps)
    nc = _CACHE[key]

    def tab_row_of(nidx):
        return ((nidx // shard) * srow + (nidx % shard)).astype(np.int32)

    w1 = np.asarray(inputs["w1"], np.float32)
    b4 = np.asarray(inputs["b4"], np.float32)
    dw1 = np.asarray(inputs["dw1"], np.float32)
    db1 = np.asarray(inputs["db1"], np.float32)
    dw1a, dw1b = dw1[:64], dw1[64:]
    c1 = b4 @ (dw1a + dw1b) + db1
    duv = np.zeros((65, 128), np.float32)
    duv[:64, :64] = dw1a
    duv[:64, 64:] = dw1b
    duv[64, :64] = c1
    iota = np.broadcast_to(np.arange(P, dtype=np.float32)[None, :], (P, P)).copy()
    ident = np.eye(P, dtype=np.float32)

    common = dict(
        x_tab=prep["x_pad"],
        w1=w1, w2=np.asarray(inputs["w2"], np.float32),
        w3=np.asarray(inputs["w3"], np.float32),
        w4=np.asarray(inputs["w4"], np.float32),
        bn_g=np.asarray(inputs["bn_gamma"], np.float32),
        bn_b=np.asarray(inputs["bn_beta"], np.float32),
        duv=duv,
        dw2=np.asarray(inputs["dw2"], np.float32),
        db2=np.asarray(inputs["db2"], np.float32).reshape(64, 1),
        dw3=np.asarray(inputs["dw3"], np.float32).reshape(64, 1),
        db3=np.asarray(inputs["db3"], np.float32).reshape(1, 1),
        iota=iota, ident=ident,
    )

    in_maps = []
    for c in range(NCORES):
        s = c * nlc
        a = tab_row_of(eli[0, s:s + nlc].astype(np.int64))
        b = tab_row_of(eli[1, s:s + nlc].astype(np.int64))
        in_maps.append(dict(
            common,
            ridx=prep["ridx"][c], colf=prep["colf"][c], nrmf=prep["nrmf"][c],
            aidx=a.reshape(-1, P).T.copy(), bidx=b.reshape(-1, P).T.copy(),
        ))
    return nc, in_maps, n_label


def kernel(**inputs):
    nc, in_maps, n_label = prepare(inputs)
    res = run_bass_kernel_spmd(nc, in_maps, core_ids=list(range(NCORES)))
    outs = [res.results[c]["out"].reshape(-1) for c in range(NCORES)]
    return np.concatenate(outs).astype(np.float32)
